# revision 39
# baseline (speedup 1.0000x reference)
"""Trainium2 Bass kernel for MHA cross-attention (nn_MHACross).

Sharding: 8 cores = 2 batches x 4 head-groups (2 heads each).
Per core (batch b, head group g):
    k,v = xmel[b] @ Wkv[g].T ; q = x[b] @ Wq[g].T ; RoPE(q, k) (scale folded
    into host-side cos/sin tables); per head scores^T = k_r @ q_r^T;
    p = exp(scores) (scores O(6), safe without max subtraction);
    out2 = v^T @ p and Z = ones^T @ (quad-summed p) on the PE;
    normalize by 1/Z (PE ones-broadcast + DVE reciprocal, no DRAM bounce);
    y_partial = attn @ Wout[:, g].T.  Host sums the 4 partials per batch.

Schedule: the exp stream on the scalar engine is the attention-phase
bottleneck, so attention for head 0 starts as soon as [k-proj h0, v-proj,
q-proj h0] finish (~38us); the h1 k/q projections run as PE filler inside
the ACT-bound h0-attention slots (with their own re-DMA'd inputs), then
the h1 attention + out-projections follow.  Z cost on the PE is quartered
by quad-summing p tiles on the DVE (bf16 2x) before the ones-matmul.
"""
import sys
sys.path.insert(0, '/opt/trn_rl_repo')
import numpy as np

DIM = 1024
NHEADS = 8
HD = 128          # head dim
HPC = 2           # heads per core
NG = 4            # head groups (cores per batch)
B, T, S = 2, 2048, 3000
NKT = DIM // 128  # contraction tiles
ROPE_BASE = 10000.0
CW = 512          # T-chunk width
PAIR = 2 * CW     # paired chunk width

_cache = {}


def _ceil_div(a, b):
    return (a + b - 1) // b


def build_nc(T=T, S=S):
    from concourse import bacc, mybir
    from concourse.tile import TileContext

    f32 = mybir.dt.float32
    bf16 = mybir.dt.bfloat16

    nc = bacc.Bacc("TRN2", target_bir_lowering=False, debug=False, num_devices=8)

    xT = nc.dram_tensor("xT", [DIM, T], bf16, kind="ExternalInput")
    xmelT = nc.dram_tensor("xmelT", [DIM, S], bf16, kind="ExternalInput")
    WqT = nc.dram_tensor("WqT", [128, NKT * HPC * HD], bf16, kind="ExternalInput")
    WkT = nc.dram_tensor("WkT", [128, NKT * HPC * HD], bf16, kind="ExternalInput")
    WvT = nc.dram_tensor("WvT", [128, NKT * HPC * HD], bf16, kind="ExternalInput")
    WoT = nc.dram_tensor("WoT", [HPC * HD, DIM], bf16, kind="ExternalInput")
    cosq = nc.dram_tensor("cosq", [HD, T], bf16, kind="ExternalInput")
    sinq = nc.dram_tensor("sinq", [HD, T], bf16, kind="ExternalInput")
    cosk = nc.dram_tensor("cosk", [HD, S], bf16, kind="ExternalInput")
    sink = nc.dram_tensor("sink", [HD, S], bf16, kind="ExternalInput")
    y = nc.dram_tensor("y", [T, DIM], bf16, kind="ExternalOutput")

    n_st = _ceil_div(S, 128)
    n_tc = _ceil_div(T, CW)
    s_big = [(i * PAIR, min(PAIR, S - i * PAIR)) for i in range(_ceil_div(S, PAIR))]
    t_big = [(i * PAIR, min(PAIR, T - i * PAIR)) for i in range(_ceil_div(T, PAIR))]
    t_chunks = [(i * CW, min(CW, T - i * CW)) for i in range(n_tc)]
    t_pairs = [t_chunks[i:i + 2] for i in range(0, n_tc, 2)]
    # Z reduction chunks: full-128 st tiles group in fours (DVE-summed),
    # ragged tail tiles stay single for the ones-matmul
    n_full = S // 128            # 23
    z_chunks = [list(range(a, min(a + 4, n_full))) for a in range(0, n_full, 4)]
    z_singles = list(range(n_full, n_st))        # [23]
    z_chunk_of_last = {c[-1]: i for i, c in enumerate(z_chunks)}

    with TileContext(nc) as tc:
        with tc.tile_pool(name="wpool", bufs=1) as wp, \
             tc.tile_pool(name="persist", bufs=1) as pp:
            wq = wp.tile([128, NKT, HPC * HD], bf16)
            wk = wp.tile([128, NKT, HPC * HD], bf16)
            wv = wp.tile([128, NKT, HPC * HD], bf16)
            wo = []
            for h in range(HPC):
                wo_h = wp.tile([128, DIM], bf16, name=f"wo{h}", uniquify=True)
                wo.append(wo_h)
            ones = wp.tile([128, 1], bf16)
            nc.vector.memset(ones[:], 1.0)
            onesrow = wp.tile([1, 128], bf16)
            nc.vector.memset(onesrow[:], 1.0)

            kT_r = [pp.tile([128, S], bf16, name=f"kT{h}", uniquify=True) for h in range(HPC)]
            qT_r = [pp.tile([128, T], bf16, name=f"qT{h}", uniquify=True) for h in range(HPC)]
            v_sb = pp.tile([128, n_st, HPC * HD], bf16)

            with tc.tile_pool(name="csP", bufs=6) as csp, \
                 tc.tile_pool(name="rtP", bufs=3) as rtp, \
                 tc.tile_pool(name="aoP", bufs=2 * HPC + 2) as aoP, \
                 tc.tile_pool(name="zP", bufs=4) as zP, \
                 tc.tile_pool(name="yP", bufs=2) as yP, \
                 tc.tile_pool(name="psA", bufs=2, space="PSUM") as psA, \
                 tc.tile_pool(name="psB", bufs=2, space="PSUM") as psB, \
                 tc.tile_pool(name="psC", bufs=2, space="PSUM") as psC:
                # weights stream on the gpsimd queue, per-kt pieces so the
                # first matmul only waits for its own 64KB
                for kt in range(NKT):
                    nc.gpsimd.dma_start(out=wk[:, kt, :], in_=WkT[:, kt * 256:(kt + 1) * 256])
                for kt in range(NKT):
                    nc.gpsimd.dma_start(out=wv[:, kt, :], in_=WvT[:, kt * 256:(kt + 1) * 256])
                for kt in range(NKT):
                    nc.gpsimd.dma_start(out=wq[:, kt, :], in_=WqT[:, kt * 256:(kt + 1) * 256])
                for h in range(HPC):
                    nc.gpsimd.dma_start(out=wo[h][:], in_=WoT[h * HD:(h + 1) * HD, :])

                def rope_phase1(ps, cs, out_sl, w):
                    # rotate-copies on ACT + the single DVE op that reads the
                    # PSUM tile; after this the PSUM slot is free
                    swp = rtp.tile([128, PAIR], f32, name="swp", tag="rt", bufs=3)
                    nc.scalar.copy(swp[0:64, :w], ps[64:128, :w])
                    nc.scalar.copy(swp[64:128, :w], ps[0:64, :w])
                    nc.vector.tensor_mul(out_sl, ps[:, :w], cs[:, :w])
                    return swp

                def rope_phase2(swp, sn, out_sl, w):
                    nc.vector.tensor_mul(swp[:, :w], swp[:, :w], sn[:, :w])
                    nc.vector.tensor_add(out_sl, out_sl, swp[:, :w])

                def load_tables(cos_d, sin_d, c0, cw):
                    cs_sb = csp.tile([128, PAIR], bf16, name="cos_sb", tag="cos", bufs=3)
                    sn_sb = csp.tile([128, PAIR], bf16, name="sin_sb", tag="sin", bufs=3)
                    nc.sync.dma_start(out=cs_sb[:, :cw], in_=cos_d[:, c0:c0 + cw])
                    nc.sync.dma_start(out=sn_sb[:, :cw], in_=sin_d[:, c0:c0 + cw])
                    return cs_sb, sn_sb

                def proj_mms(psl, w_sb, heads, src_fn, cw):
                    # kt-outer matmuls into [128, PAIR] psum tiles per head
                    halves = [(o, min(CW, cw - o)) for o in range(0, cw, CW)]
                    for kt in range(NKT):
                        for hi, h in enumerate(heads):
                            for (o, ow) in halves:
                                nc.tensor.matmul(
                                    psl[hi][:, o:o + ow],
                                    w_sb[:, kt, h * HD:(h + 1) * HD],
                                    src_fn(kt, o, ow),
                                    start=(kt == 0), stop=(kt == NKT - 1),
                                    skip_group_check=True)

                def rope_both(psl, cs_sb, sn_sb, outs, cw):
                    swps = [rope_phase1(psl[hi], cs_sb, outs[hi], cw)
                            for hi in range(len(psl))]
                    for hi in range(len(psl)):
                        rope_phase2(swps[hi], sn_sb, outs[hi], cw)

                def rope_dve_only(ps, cs_sb, sn_sb, out_sl, w):
                    # for fillers inside the exp-bound attention phase: keep
                    # the scalar engine free, do the rotate as half-slice muls
                    swp = rtp.tile([128, PAIR], f32, name="swp", tag="rt", bufs=3)
                    nc.vector.tensor_mul(swp[0:64, :w], ps[64:128, :w], sn_sb[0:64, :w])
                    nc.vector.tensor_mul(swp[64:128, :w], ps[0:64, :w], sn_sb[64:128, :w])
                    nc.vector.tensor_mul(out_sl, ps[:, :w], cs_sb[:, :w])
                    nc.vector.tensor_add(out_sl, out_sl, swp[:, :w])

                # ------------- pre-phase: h0 projections + all of v ----------
                with tc.tile_pool(name="xpool", bufs=1) as xp:
                    xm = [xp.tile([128, S], bf16, name=f"xm{kt}", uniquify=True)
                          for kt in range(NKT)]
                    xq = [xp.tile([128, T], bf16, name=f"xq{kt}", uniquify=True)
                          for kt in range(NKT)]

                    def emit_scp(s0, sw):
                        # pieces before tables: tables are only needed at rope
                        # time, ~10us after the first matmul
                        for kt in range(NKT):
                            nc.sync.dma_start(out=xm[kt][:, s0:s0 + sw],
                                              in_=xmelT[kt * 128:(kt + 1) * 128, s0:s0 + sw])
                        cs_sb, sn_sb = load_tables(cosk, sink, s0, sw)
                        kps = [psA.tile([128, PAIR], f32, name="kps", tag="sc", bufs=2)]
                        proj_mms(kps, wk, [0],
                                 lambda kt, o, ow: xm[kt][:, s0 + o:s0 + o + ow], sw)
                        rope_both(kps, cs_sb, sn_sb, [kT_r[0][:, s0:s0 + sw]], sw)
                        st_lo, st_hi = s0 // 128, _ceil_div(s0 + sw, 128)
                        for st0 in range(st_lo, st_hi, 2):
                            sts = [st for st in (st0, st0 + 1) if st < st_hi]
                            vps = psB.tile([128, 2 * HPC * HD], f32, name="vps", tag="acc", bufs=2)
                            for vi, st in enumerate(sts):
                                t0 = st * 128
                                scnt = min(128, S - t0)
                                for kt in range(NKT):
                                    nc.tensor.matmul(
                                        vps[:scnt, vi * 256:(vi + 1) * 256],
                                        xm[kt][:, t0:t0 + scnt],
                                        wv[:, kt, :],
                                        start=(kt == 0), stop=(kt == NKT - 1),
                                        skip_group_check=True)
                            if len(sts) == 2 and min(128, S - sts[-1] * 128) == 128:
                                nc.scalar.copy(v_sb[:, st0:st0 + 2, :], vps[:, :])
                            else:
                                for vi, st in enumerate(sts):
                                    scnt = min(128, S - st * 128)
                                    nc.scalar.copy(v_sb[:scnt, st, :],
                                                   vps[:scnt, vi * 256:(vi + 1) * 256])

                    def emit_qcp(c0, cw):
                        for kt in range(NKT):
                            nc.sync.dma_start(out=xq[kt][:, c0:c0 + cw],
                                              in_=xT[kt * 128:(kt + 1) * 128, c0:c0 + cw])
                        cs_sb, sn_sb = load_tables(cosq, sinq, c0, cw)
                        qps = [psA.tile([128, PAIR], f32, name="qps", tag="sc", bufs=2)]
                        proj_mms(qps, wq, [0],
                                 lambda kt, o, ow: xq[kt][:, c0 + o:c0 + o + ow], cw)
                        rope_both(qps, cs_sb, sn_sb, [qT_r[0][:, c0:c0 + cw]], cw)

                    emit_scp(*s_big[0])
                    emit_qcp(*t_big[0])
                    emit_scp(*s_big[1])
                    emit_qcp(*t_big[1])
                    emit_scp(*s_big[2])

                # ------------- attention (+ h1-proj fillers) -----------------
                with tc.tile_pool(name="pP", bufs=28) as pP, \
                     tc.tile_pool(name="ppairP", bufs=10) as ppP, \
                     tc.tile_pool(name="fxP", bufs=24) as fxp:

                    # h1 k/q projection fillers: re-DMA inputs into a small
                    # pool, prefetched one filler ahead
                    def make_filler(kind, c0, cw):
                        st_ = {}

                        def dma():
                            if kind == "k":
                                st_["cs"], st_["sn"] = load_tables(cosk, sink, c0, cw)
                                srcT = xmelT
                            else:
                                st_["cs"], st_["sn"] = load_tables(cosq, sinq, c0, cw)
                                srcT = xT
                            tiles = []
                            for kt in range(NKT):
                                fx = fxp.tile([128, CW], bf16, name="fx", tag="fx", bufs=24)
                                nc.sync.dma_start(out=fx[:, :cw],
                                                  in_=srcT[kt * 128:(kt + 1) * 128, c0:c0 + cw])
                                tiles.append(fx)
                            st_["tiles"] = tiles

                        def compute():
                            w_sb = wk if kind == "k" else wq
                            out = (kT_r if kind == "k" else qT_r)[1][:, c0:c0 + cw]
                            ps = [psA.tile([128, PAIR], f32, name="fps", tag="sc", bufs=2)]
                            proj_mms(ps, w_sb, [1],
                                     lambda kt, o, ow: st_["tiles"][kt][:, o:o + ow], cw)
                            rope_dve_only(ps[0], st_["cs"], st_["sn"], out, cw)

                        return (dma, compute)

                    halves = []
                    for (s0, sw) in s_big:
                        for o in range(0, sw, CW):
                            halves.append(("k", s0 + o, min(CW, sw - o)))
                    for (c0, cw) in t_big:
                        for o in range(0, cw, CW):
                            halves.append(("q", c0 + o, min(CW, cw - o)))
                    fillers = [make_filler(*h) for h in halves]   # 6 k + 4 q
                    # one ~1.8us unit per h0-half slot (h1 starts at slot 10);
                    # q-cp1 (units 8,9) is only needed by slot 15
                    filler_slots = {1: [0], 2: [1], 3: [2], 4: [3], 5: [4],
                                    6: [5], 7: [6], 8: [7], 9: [8, 9]}

                    # last group of each block is tiny so the final exp
                    # gates as little serial tail work as possible
                    gplan = [(0, 6), (6, 6), (12, 6), (18, 4), (22, 2)]
                    groups = []
                    for h in range(HPC):
                        for pi in range(len(t_pairs)):
                            for (g0, gc) in gplan:
                                groups.append((pi, h, g0, gc))
                    DELAY = 3
                    blocks = {}

                    def emit_sc_exp(key):
                        pi, h, g0, gc = key
                        pair = t_pairs[pi]
                        pw = sum(cw for _, cw in pair)
                        bk = blocks.setdefault((pi, h), {"ptiles": {}, "pairs": {}})
                        for st in range(g0, g0 + gc):
                            s0 = st * 128
                            scnt = min(128, S - s0)
                            scps = psA.tile([128, PAIR], f32, name="scps", tag="sc", bufs=2)
                            for ci, (c0, cw) in enumerate(pair):
                                nc.tensor.matmul(
                                    scps[:scnt, ci * CW: ci * CW + cw],
                                    kT_r[h][:, s0:s0 + scnt],
                                    qT_r[h][:, c0:c0 + cw],
                                    start=True, stop=True,
                                    skip_group_check=True)
                            p_t = pP.tile([128, PAIR], bf16, name="p_t", tag="p", bufs=28)
                            nc.scalar.activation(p_t[:scnt, :pw], scps[:scnt, :pw],
                                                 mybir.ActivationFunctionType.Exp)
                            bk["ptiles"][st] = (p_t, scnt)
                            # chunked p sums (DVE, bf16 2x) feed the Z matmuls
                            if st < n_full:
                                if st % 4 == 0:
                                    bk["zacc"] = p_t
                                else:
                                    pp_t = ppP.tile([128, PAIR], bf16, name="pp_t", tag="pp", bufs=10)
                                    nc.vector.tensor_add(pp_t[:, :pw], bk["zacc"][:, :pw],
                                                         p_t[:, :pw])
                                    bk["zacc"] = pp_t
                                if st in z_chunk_of_last:
                                    bk["pairs"][z_chunk_of_last[st]] = bk["zacc"]

                    def emit_zav(key):
                        pi, h, g0, gc = key
                        pair = t_pairs[pi]
                        bk = blocks[(pi, h)]
                        last = (g0 + gc == n_st)
                        if g0 == 0:
                            # ci=0 lives at partition 0, ci=1 at partition 32
                            bk["zps"] = psC.tile([64, CW], f32, name="zps", tag="z", bufs=2)
                            bk["o2"] = [psB.tile([128, CW], f32, name="o2ps", tag="acc", bufs=2)
                                        for _ in pair]
                        sts = list(range(g0, g0 + gc))
                        for cki, ck in enumerate(z_chunks):
                            if ck[-1] in sts:
                                pp_t = bk["pairs"][cki]
                                first = (cki == 0)
                                for ci, (c0, cw) in enumerate(pair):
                                    nc.tensor.matmul(
                                        bk["zps"][32 * ci:32 * ci + 1, :cw],
                                        ones[:128, :],
                                        pp_t[:, ci * CW: ci * CW + cw],
                                        start=first, stop=False,
                                        skip_group_check=True)
                        if last:
                            for si, st in enumerate(z_singles):
                                p_t, scnt = bk["ptiles"][st]
                                stop = (si == len(z_singles) - 1)
                                for ci, (c0, cw) in enumerate(pair):
                                    nc.tensor.matmul(
                                        bk["zps"][32 * ci:32 * ci + 1, :cw],
                                        ones[:scnt, :],
                                        p_t[:scnt, ci * CW: ci * CW + cw],
                                        start=False, stop=stop,
                                        skip_group_check=True)
                            # stage Z rows to SBUF as bf16 (single cast-copy)
                            bk["zrb"] = []
                            for ci, (c0, cw) in enumerate(pair):
                                zrb = zP.tile([1, CW], bf16, name="zrb", tag="zrb", bufs=4)
                                nc.vector.tensor_copy(zrb[:, :], bk["zps"][32 * ci:32 * ci + 1, :])
                                bk["zrb"].append(zrb)
                        for st in sts:
                            p_t, scnt = bk["ptiles"][st]
                            for ci, (c0, cw) in enumerate(pair):
                                nc.tensor.matmul(
                                    bk["o2"][ci][:, :cw],
                                    v_sb[:scnt, st, h * HD:(h + 1) * HD],
                                    p_t[:scnt, ci * CW: ci * CW + cw],
                                    start=(st == 0), stop=(st == n_st - 1))
                        if last:
                            # broadcast Z with a ones-column matmul, reciprocal
                            # on the [128,CW] tile, then normalize
                            bk["ao"] = []
                            for ci, (c0, cw) in enumerate(pair):
                                zrp = psC.tile([128, CW], f32, name="zrp", tag="z", bufs=2)
                                nc.tensor.matmul(zrp[:, :], onesrow[:, :], bk["zrb"][ci][:, :],
                                                 start=True, stop=True,
                                                 skip_group_check=True)
                                zrep = zP.tile([128, CW], f32, name="zrep", tag="zrep", bufs=4)
                                nc.vector.reciprocal_approx_fast(out=zrep[:, :], in_=zrp[:, :])
                                ao_h = aoP.tile([128, CW], bf16, name="ao", tag="ao", bufs=2 * HPC + 2)
                                nc.vector.tensor_mul(ao_h[:, :cw], bk["o2"][ci][:, :cw],
                                                     zrep[:, :cw])
                                bk["ao"].append(ao_h)

                    def emit_outproj(pi, copies="dve"):
                        pair = t_pairs[pi]
                        for ci, (c0, cw) in enumerate(pair):
                            for tt in range(cw // 128):
                                y_sb = yP.tile([128, DIM], bf16, name="y_sb", tag="ysb", bufs=2)
                                for nn in range(DIM // 512):
                                    yps = psA.tile([128, 512], f32, name="yps", tag="sc", bufs=2)
                                    for h in range(HPC):
                                        nc.tensor.matmul(
                                            yps[:],
                                            blocks[(pi, h)]["ao"][ci][:, tt * 128:(tt + 1) * 128],
                                            wo[h][:, nn * 512:(nn + 1) * 512],
                                            start=(h == 0), stop=(h == HPC - 1))
                                    on_act = copies == "act" or (copies == "alt" and nn % 2 == 0)
                                    if on_act:
                                        nc.scalar.copy(y_sb[:, nn * 512:(nn + 1) * 512], yps[:])
                                    else:
                                        nc.vector.tensor_copy(y_sb[:, nn * 512:(nn + 1) * 512], yps[:])
                                nc.sync.dma_start(out=y[c0 + tt * 128: c0 + (tt + 1) * 128, :], in_=y_sb[:])

                    # both out-projections run in the drain (PE-only work
                    # once the exps are done); the final one stages its y
                    # copies on the by-then-idle scalar engine
                    for fi in filler_slots.get(1, []):
                        fillers[fi][0]()                 # prefetch first DMA
                    for i, key in enumerate(groups):
                        emit_sc_exp(key)
                        if i >= DELAY:
                            emit_zav(groups[i - DELAY])
                        for fi in filler_slots.get(i, []):
                            fillers[fi][1]()             # filler compute
                        for fi in filler_slots.get(i + 1, []):
                            fillers[fi][0]()             # prefetch dma
                    # drain: run the exp-ready zavs first, then outproj(pi0)
                    # fills the wait for the final group's exps (its copies
                    # land on ACT right after the last exp, keeping the DVE
                    # free for the last block's 1/Z chain)
                    nblk = len(groups)
                    emit_zav(groups[nblk - 3])
                    emit_zav(groups[nblk - 2])
                    emit_outproj(0, copies="act")
                    emit_zav(groups[nblk - 1])
                    emit_outproj(1, copies="alt")

    nc.compile()
    return nc


def _host_tables(T=T, S=S):
    import ml_dtypes
    scale = float(HD) ** (-0.25)
    inv = 1.0 / (ROPE_BASE ** (np.arange(0, HD, 2, dtype=np.float64) / HD))  # [64]

    def tables(L):
        fr = np.outer(inv, np.arange(L, dtype=np.float64))  # [64, L]
        c = np.cos(fr) * scale
        s = np.sin(fr) * scale
        cos = np.concatenate([c, c], axis=0).astype(ml_dtypes.bfloat16)
        sin = np.concatenate([-s, s], axis=0).astype(ml_dtypes.bfloat16)
        return np.ascontiguousarray(cos), np.ascontiguousarray(sin)

    cosq_, sinq_ = tables(T)
    cosk_, sink_ = tables(S)
    return cosq_, sinq_, cosk_, sink_


def make_in_maps(x, xmel, Wq, Wkv, Wout):
    import ml_dtypes
    bf = ml_dtypes.bfloat16
    Bx, Tx, C = x.shape
    Sx = xmel.shape[1]
    cosq_, sinq_, cosk_, sink_ = _host_tables(Tx, Sx)

    x = np.asarray(x, dtype=np.float32)
    xmel = np.asarray(xmel, dtype=np.float32)
    Wq = np.asarray(Wq, dtype=np.float32)
    Wkv = np.asarray(Wkv, dtype=np.float32)
    Wout = np.asarray(Wout, dtype=np.float32)

    xT_b = [np.ascontiguousarray(x[b].T).astype(bf) for b in range(Bx)]
    xmelT_b = [np.ascontiguousarray(xmel[b].T).astype(bf) for b in range(Bx)]
    gsz = HPC * HD  # 256
    WqT_g, WkT_g, WvT_g, WoT_g = [], [], [], []
    for g in range(NG):
        r0 = g * gsz
        def prearr(wt):  # [DIM, gsz] -> [128, NKT*gsz], row p holds [kt, n]
            return np.ascontiguousarray(
                wt.reshape(NKT, 128, gsz).transpose(1, 0, 2).reshape(128, NKT * gsz)).astype(bf)
        WqT_g.append(prearr(Wq[r0:r0 + gsz, :].T))
        WkT_g.append(prearr(Wkv[r0:r0 + gsz, :].T))
        WvT_g.append(prearr(Wkv[DIM + r0:DIM + r0 + gsz, :].T))
        WoT_g.append(np.ascontiguousarray(Wout[:, r0:r0 + gsz].T).astype(bf))

    in_maps = []
    for c in range(Bx * NG):
        b, g = c // NG, c % NG
        in_maps.append({
            "xT": xT_b[b], "xmelT": xmelT_b[b],
            "WqT": WqT_g[g], "WkT": WkT_g[g], "WvT": WvT_g[g], "WoT": WoT_g[g],
            "cosq": cosq_, "sinq": sinq_, "cosk": cosk_, "sink": sink_,
        })
    return in_maps


def kernel(x, xmel, Wq, Wkv, Wout):
    from concourse.bass_utils import run_bass_kernel_spmd

    x = np.asarray(x, dtype=np.float32)
    xmel = np.asarray(xmel, dtype=np.float32)
    Bx, Tx, C = x.shape
    Sx = xmel.shape[1]
    assert (Bx, Tx, C, Sx) == (B, T, DIM, S)

    if "nc" not in _cache:
        _cache["nc"] = build_nc()
    nc = _cache["nc"]

    in_maps = make_in_maps(x, xmel,
                           np.asarray(Wq, dtype=np.float32),
                           np.asarray(Wkv, dtype=np.float32),
                           np.asarray(Wout, dtype=np.float32))
    res = run_bass_kernel_spmd(nc, in_maps, list(range(8)))
    out = np.zeros((B, T, DIM), dtype=np.float32)
    for c in range(8):
        b = c // NG
        out[b] += np.asarray(res.results[c]["y"], dtype=np.float32)
    return out
